# revision 1
# baseline (speedup 1.0000x reference)
"""Trainium2 Bass kernel for DifferentiableMaxMedian (5x5 reflect-padded
max filter + soft-median filter, per-channel mix).

Contract: kernel(**inputs) takes FULL numpy inputs
  x: (4,32,256,256) f32, mix: (1,32,1,1) f32, beta_raw: () f32
and returns the full (4,32,256,256) f32 output.

Sharding: pure data parallel over B*C = 128 (b,c) slices -> 16 slices/core
across 8 cores.

Per-core layout: each slice is reflect-padded host-side to 260x260 and cut
into 32 row-strips of R=8 output rows. 4 slices x 32 strips = 128 SBUF
partitions per big tile (4 big tiles/core). A partition's free dim holds its
strip's 12 rows (8 + 4 halo) x 260 padded cols, fully contiguous in DRAM, so
each big tile loads with ONE DMA and every 5x5 window tap is a pure free-dim
AP offset.

Math: s_k = exp(C - beta*|v_k - mu|); the softmax ratio is invariant to C.
S = sum_k s_k and T = sum_k s_k*v_k accumulate on the TensorEngine as
identity-matmul PSUM accumulations (half-tiles of 4 rows so S/T double-buffer
within the 8 PSUM banks). out = lam*(T/S) + (1-lam)*max5x5.
"""

import numpy as np

from concourse import bass
import concourse.mybir as mybir
import concourse.tile_sem_assignment as _tsa
from concourse.ap import AP
from concourse.bass_utils import run_bass_kernel_spmd
from concourse.tile import TileContext
from concourse.tile_rust import add_dep_helper
from concourse.mybir import AluOpType as ALU
from concourse.mybir import ActivationFunctionType as AF

# All our DMAs issue from the sync engine's single HW-DGE ring (FIFO
# completion), so one bookkeeping sem lane is sound — and it keeps
# per-instruction wait counts under the per-engine ISA limits.
_tsa.NUM_HWDGE_SEMS = 1

F32 = mybir.dt.float32

B, C, H, W = 4, 32, 256, 256
NCORES = 8
SL = (B * C) // NCORES     # 16 slices per core
R = 8                      # output rows per strip
RP = R + 4                 # rows incl halo
WP = W + 4                 # padded width
NSTRIP = H // R            # 32 strips per slice
SPT = 128 // NSTRIP        # 4 slices per big tile
NT = SL // SPT             # 4 big tiles per core
HR = R // 2                # rows per half-tile (PSUM double-buffer unit)
# exp stability bias (cancels in the softmax ratio). Keeps S = sum_k
# exp(C - beta*|d_k|) well inside ACT-Ln's valid range (< 2^64): with the
# given inputs beta*min_k|d_k| <= ~10, so S is in ~[e^30, 25*e^40].
C_BIAS = 40.0


def _dram_ap(t, offset, dims):
    return AP(tensor=t[:].tensor, offset=offset, ap=dims)


def _elide_covered_waits(nc):
    """Drop sem waits already covered by an earlier wait on the same engine.

    Engines execute their instruction streams in order and Tile semaphores
    only increase, so a wait for (sem, value) when an earlier instruction on
    the same engine already waited for (sem, value' >= value) is a no-op.
    Tile's sem assignment emits these redundant waits at PSUM slot-reuse
    boundaries, where they overflow the LDWEIGHTS format's 1-wait budget.
    """
    skip = ("InstISA", "InstCustomDveAnt", "InstEventSemaphore")
    for b in nc.m.functions[0].blocks:
        seen = {}
        for ins in b.instructions:
            si = ins.sync_info
            if si is None or type(ins).__name__ in skip:
                continue
            w = si.on_wait
            if not w:
                continue
            eng = str(ins.engine).split(".")[-1]
            em = seen.setdefault(eng, {})
            kept = []
            for x in w:
                monotone = x.ant_name is not None and x.ant_name.startswith(
                    ("PE_", "DVE_", "Activation_", "Pool_", "SP_",
                     "DMAHW", "DMASW"))
                if (x.wait_mode != "sem-ge-imm" or x.wait_value is None
                        or x.wait_reg is not None or not monotone):
                    kept.append(x)
                    continue
                # ACT's same-engine self-waits are redundant (in-order
                # engine, writes cannot overtake) — but only drop them when
                # the instruction carries other waits too, to stay within
                # the 1-wait format budget without tripping CoreSim's
                # strict same-engine RAW detector on solo self-waits.
                if (eng == "Activation" and len(w) > 1
                        and x.ant_name.startswith(eng + "_")):
                    continue
                if em.get(x.ant_name, -1) >= x.wait_value:
                    continue
                kept.append(x)
                em[x.ant_name] = x.wait_value
            if len(kept) != len(w):
                si.on_wait = kept


def _split_excess_waits(nc):
    """Move all-but-one sync waits onto injected same-engine NoOps.

    TPB compute instruction formats encode a single embedded wait command
    (walrus rejects more with "Too many sync wait commands"). A NoOp on the
    same in-order engine carrying the extra waits immediately before the
    instruction is semantically identical: the engine stalls at the nop
    until the semaphores reach their thresholds, then proceeds.
    """
    n = 0
    skip = ("InstISA", "InstCustomDveAnt", "InstEventSemaphore")
    for b in nc.m.functions[0].blocks:
        out = []
        changed = False
        for ins in b.instructions:
            si = ins.sync_info
            w = list(si.on_wait) if si is not None and si.on_wait else []
            if len(w) > 1 and type(ins).__name__ not in skip:
                for x in w[:-1]:
                    n += 1
                    nop = mybir.InstNoOp(name=f"I-waitnop-{n}",
                                         engine=ins.engine)
                    nop.sync_info = mybir.SyncInfo(on_wait=[x], on_update=[])
                    try:
                        nop.debug = ins.debug
                    except Exception:
                        pass
                    out.append(nop)
                si.on_wait = [w[-1]]
                changed = True
            out.append(ins)
        if changed:
            b.instructions = out


def build_program(beta: float, split_waits: bool = True):
    nc = bass.Bass()
    xs = nc.declare_dram_parameter("xs", [SL, H + 4, W + 4], F32, isOutput=False)
    lam = nc.declare_dram_parameter("lam", [NT, 128, 1], F32, isOutput=False)
    ident = nc.declare_dram_parameter("ident", [128, 128], F32, isOutput=False)
    zeros = nc.declare_dram_parameter("zeros", [128, 512], F32, isOutput=False)
    y = nc.declare_dram_parameter("y", [SL, H, W], F32, isOutput=True)

    HW = H * W
    HPWP = (H + 4) * WP

    with TileContext(nc) as tc:
        with (
            tc.tile_pool(name="consts", bufs=1) as cpool,
            tc.tile_pool(name="main", bufs=2) as pool,
            tc.tile_pool(name="single", bufs=1) as spool,
            tc.psum_pool(name="ps", bufs=2) as pspool,
        ):
            itile = cpool.tile([128, 128], F32)
            nc.sync.dma_start(out=itile[:], in_=ident[:])
            ztile = cpool.tile([128, 512], F32)
            nc.sync.dma_start(out=ztile[:], in_=zeros[:])
            cbias = cpool.tile([128, 1], F32)
            nc.vector.memset(cbias[:], C_BIAS)

            psum_releases = {}
            for t in range(NT):
                # ---- load big tile t (4 slices x 32 strips) in one DMA ----
                X = pool.tile([128, RP, WP], F32, tag="X")
                nc.sync.dma_start(
                    out=X[:],
                    in_=_dram_ap(xs, t * SPT * HPWP,
                                 [[HPWP, SPT], [R * WP, NSTRIP], [1, RP * WP]]),
                )
                lamt = pool.tile([128, 1], F32, tag="lam")
                nc.sync.dma_start(out=lamt[:], in_=lam[t])

                # ---- separable 25-sum (for the mean) on DVE ----
                V = spool.tile([128, R, WP], F32, tag="V")
                nc.vector.tensor_tensor(V[:], X[:, 0:R, :], X[:, 1:R + 1, :], ALU.add)
                for dy in (2, 3, 4):
                    nc.vector.tensor_tensor(V[:], V[:], X[:, dy:dy + R, :], ALU.add)
                M = spool.tile([128, R, W], F32, tag="M")
                nc.vector.tensor_tensor(M[:], V[:, :, 0:W], V[:, :, 1:W + 1], ALU.add)
                for dx in (2, 3, 4):
                    nc.vector.tensor_tensor(M[:], M[:], V[:, :, dx:dx + W], ALU.add)

                # ---- separable 5x5 max (DVE) ----
                MV = spool.tile([128, R, WP], F32, tag="MV")
                nc.vector.tensor_tensor(MV[:], X[:, 0:R, :], X[:, 1:R + 1, :], ALU.max)
                for dy in (2, 3, 4):
                    nc.vector.tensor_tensor(MV[:], MV[:], X[:, dy:dy + R, :], ALU.max)
                MX = spool.tile([128, R, W], F32, tag="MX")
                nc.vector.tensor_tensor(MX[:], MV[:, :, 0:W], MV[:, :, 1:W + 1], ALU.max)
                for dx in (2, 3, 4):
                    nc.vector.tensor_tensor(MX[:], MX[:], MV[:, :, dx:dx + W], ALU.max)

                # ---- 25 taps: softmax-weighted sums on DVE/ACT/PE ----
                # Processed in two half-tiles (rows 0-3 / 4-7 per strip) so
                # the S/T PSUM accumulators take 2 banks each and
                # double-buffer (2 tags x 2 bufs x 2 banks = 8 banks). Each
                # accumulation group is opened by a zeros-rhs "warmup"
                # matmul that alone carries the PSUM slot-reuse drain wait
                # (the LDWEIGHTS format allows one sync wait only).
                for h in range(2):
                    r0 = h * HR
                    g = 2 * t + h
                    S_ps = pspool.tile([128, HR, W], F32, tag="S")
                    T_ps = pspool.tile([128, HR, W], F32, tag="T")
                    # A PE nop (CTRL format: multi-wait budget) syncs on the
                    # recycled slot's DVE readers so the warmup matmuls keep
                    # only their single PE-drain wait.
                    if g >= 2:
                        pe_nop = nc.tensor.nop()
                        for rel in psum_releases[g - 2]:
                            add_dep_helper(pe_nop.ins, rel.ins, sync=True,
                                           reason="psum slot reader sync")
                    else:
                        pe_nop = None
                    for cch in range(2):
                        sel = (slice(None), slice(2 * cch, 2 * cch + 2),
                               slice(None))
                        wm_t = nc.tensor.matmul(T_ps[sel], itile[:], ztile[:],
                                                start=True, stop=False)
                        wm_s = nc.tensor.matmul(S_ps[sel], itile[:], ztile[:],
                                                start=True, stop=False)
                        if pe_nop is not None:
                            add_dep_helper(wm_t.ins, pe_nop.ins, sync=True,
                                           reason="nop before warmup")
                            add_dep_helper(wm_s.ins, pe_nop.ins, sync=True,
                                           reason="nop before warmup")
                    ntap = 0
                    for dy in range(5):
                        for dx in range(5):
                            v_ap = X[:, r0 + dy:r0 + dy + HR, dx:dx + W]
                            d = pool.tile([128, HR, W], F32, tag="d")
                            # d = M*(-1/25) + v_k
                            nc.vector.scalar_tensor_tensor(
                                d[:], M[:, r0:r0 + HR, :], -1.0 / 25.0, v_ap,
                                ALU.mult, ALU.add)
                            # a = |d| in place: clear the f32 sign bit via
                            # an int32 bitcast (valid TS op, 2x mode)
                            di = d[:].bitcast(mybir.dt.int32)
                            nc.vector.tensor_scalar(di, di, 0x7FFFFFFF, None,
                                                    ALU.bitwise_and)
                            st = pool.tile([128, HR, W], F32, tag="st")
                            nc.scalar.activation(st[:], d[:], AF.Exp,
                                                 bias=cbias[:], scale=-beta)
                            pt = pool.tile([128, HR, W], F32, tag="pt")
                            nc.vector.tensor_tensor(pt[:], st[:], v_ap,
                                                    ALU.mult)
                            last = ntap == 24
                            for cch in range(2):
                                sel = (slice(None),
                                       slice(2 * cch, 2 * cch + 2),
                                       slice(None))
                                nc.tensor.matmul(T_ps[sel], itile[:], pt[sel],
                                                 start=False, stop=last)
                                nc.tensor.matmul(S_ps[sel], itile[:], st[sel],
                                                 start=False, stop=last)
                            ntap += 1

                    # ---- combine: out = lam*(T/S) + (1-lam)*max ----
                    # 1/S = exp(-ln S) on ACT (standard opcodes; Ln and Exp
                    # share the natural_log_exp table set). S spans
                    # ~[1e27, 3e31] here so both stay in range.
                    MXh = MX[:, r0:r0 + HR, :]
                    lnS = spool.tile([128, HR, W], F32, tag="lnS")
                    rel_s = nc.scalar.activation(lnS[:], S_ps[:], AF.Ln)
                    rS = spool.tile([128, HR, W], F32, tag="rS")
                    nc.scalar.activation(rS[:], lnS[:], AF.Exp, scale=-1.0)
                    med = spool.tile([128, HR, W], F32, tag="med")
                    rel_t = nc.vector.tensor_tensor(med[:], rS[:], T_ps[:],
                                                    ALU.mult)
                    psum_releases[g] = (rel_s, rel_t)
                    nc.vector.tensor_tensor(med[:], med[:], MXh, ALU.subtract)
                    out_t = pool.tile([128, HR, W], F32, tag="out")
                    nc.vector.scalar_tensor_tensor(
                        out_t[:], med[:], lamt[:], MXh, ALU.mult, ALU.add)

                    # ---- store half-tile ----
                    nc.sync.dma_start(
                        out=_dram_ap(y, t * SPT * HW + h * HR * W,
                                     [[HW, SPT], [R * W, NSTRIP],
                                      [1, HR * W]]),
                        in_=out_t[:],
                    )
    _elide_covered_waits(nc)
    if split_waits:
        # Mechanical transform for walrus's 1-wait instruction formats;
        # skip under CoreSim (its race detector requires sem updates on
        # every instruction, which the injected bare NoOps lack).
        _split_excess_waits(nc)
    return nc


def _make_inputs(x, mix, beta_raw):
    """Host-side sharding. Returns (beta, in_maps)."""
    x = np.ascontiguousarray(x, dtype=np.float32)
    mix = np.asarray(mix, dtype=np.float32).reshape(C)
    beta_raw = float(np.asarray(beta_raw, dtype=np.float32))
    beta = float(5.0 + 45.0 / (1.0 + np.exp(-beta_raw)))
    lam_c = (1.0 / (1.0 + np.exp(-mix.astype(np.float64)))).astype(np.float32)

    xs_all = np.pad(x.reshape(B * C, H, W), ((0, 0), (2, 2), (2, 2)),
                    mode="reflect")
    ident = np.eye(128, dtype=np.float32)
    zeros = np.zeros((128, 512), dtype=np.float32)
    in_maps = []
    for core in range(NCORES):
        sl0 = core * SL
        shard = np.ascontiguousarray(xs_all[sl0:sl0 + SL])
        lam_t = np.empty((NT, 128, 1), dtype=np.float32)
        for t in range(NT):
            for p in range(128):
                g_slice = sl0 + t * SPT + p // NSTRIP
                lam_t[t, p, 0] = lam_c[g_slice % C]
        in_maps.append({"xs": shard, "lam": lam_t, "ident": ident,
                        "zeros": zeros})
    return beta, in_maps


def kernel(x, mix, beta_raw):
    beta, in_maps = _make_inputs(x, mix, beta_raw)
    nc = build_program(beta)
    res = run_bass_kernel_spmd(nc, in_maps, list(range(NCORES))).results
    out = np.concatenate([res[i]["y"].reshape(SL, H, W) for i in range(NCORES)],
                         axis=0)
    return np.ascontiguousarray(out.reshape(B, C, H, W))



# revision 14
# speedup vs baseline: 1.3968x; 1.3968x over previous
"""Trainium2 Bass kernel for DifferentiableMaxMedian (5x5 reflect-padded
max filter + soft-median filter, per-channel mix).

Contract: kernel(**inputs) takes FULL numpy inputs
  x: (4,32,256,256) f32, mix: (1,32,1,1) f32, beta_raw: () f32
and returns the full (4,32,256,256) f32 output.

Sharding: pure data parallel over B*C = 128 (b,c) slices -> 16 slices/core
across 8 cores.

Per-core layout: each slice is reflect-padded host-side to 260x260 and cut
into 32 row-strips of R=8 output rows. 4 slices x 32 strips = 128 SBUF
partitions per big tile (4 big tiles/core). A partition's free dim holds its
strip's 12 rows (8 + 4 halo) x 260 padded cols, fully contiguous in DRAM, so
each big tile loads with ONE DMA and every 5x5 window tap is a pure free-dim
AP offset.

v2 (bf16 rework): the 25-tap softmax pipeline runs in bf16 so DVE
tensor_tensor hits 2x_1P mode and PE matmuls run at bf16 rate (~4x the fp32
baseline). The 5 dy-taps of each dx column are merged into single 5120-elem
instructions (one d-add, one sign-mask abs, one ACT exp, one pt-mult per
(dx,half)), amortizing per-instruction overheads. Work is spread over four
engines: DVE (d-adds, some abs/pt), ACT (exp, some abs, f32->bf16 converts),
Pool (5x5 max filter + some pt-mults), PE (S/T tap accumulation via bf16
identity matmuls). The mean filter stays in f32 on DVE for accuracy. The
per-channel lambda is folded into the combine as Exp(-lnS + ln(lam)) = lam/S
(ACT bias) and a (1-lam)*max precomputed per tile.

Math: s_k = exp(C - beta*|v_k - mu|); the softmax ratio is invariant to C.
S = sum_k s_k and T = sum_k s_k*v_k accumulate on the TensorEngine as
identity-matmul PSUM accumulations (half-tiles of 4 rows so S/T double-buffer
within the 8 PSUM banks). out = lam*(T/S) + (1-lam)*max5x5.
"""

import numpy as np

from concourse import bass
import concourse.mybir as mybir
import concourse.tile_sem_assignment as _tsa
from concourse.ap import AP
from concourse.bass_utils import run_bass_kernel_spmd
from concourse.tile import TileContext
from concourse.tile_rust import add_dep_helper
from concourse.mybir import AluOpType as ALU
from concourse.mybir import ActivationFunctionType as AF

# All our DMAs issue from the sync engine's single HW-DGE ring (FIFO
# completion), so one bookkeeping sem lane is sound — and it keeps
# per-instruction wait counts under the per-engine ISA limits.
_tsa.NUM_HWDGE_SEMS = 1

F32 = mybir.dt.float32
BF16 = mybir.dt.bfloat16
I32 = mybir.dt.int32

B, C, H, W = 4, 32, 256, 256
NCORES = 8
SL = (B * C) // NCORES     # 16 slices per core
R = 8                      # output rows per strip
RP = R + 4                 # rows incl halo
WP = W + 4                 # padded width
NSTRIP = H // R            # 32 strips per slice
SPT = 128 // NSTRIP        # 4 slices per big tile
NT = SL // SPT             # 4 big tiles per core
HR = R // 2                # rows per half-tile (PSUM double-buffer unit)
# exp stability bias (cancels in the softmax ratio). Keeps S = sum_k
# exp(C - beta*|d_k|) well inside f32/bf16 range: with the given inputs
# beta*min_k|d_k| <= ~10, so S is in ~[e^30, 25*e^40].
C_BIAS = 40.0

# Work-split knobs (which dx columns run where), tuned from trace engine
# occupancy. abs on ACT for these dx; pt-mult on Pool for these dx.
# (Pool's Q7 ucode supports TT add/mult but NOT max, so the 5x5 max filter
# stays on DVE and Pool takes most of the pt-mult stream instead.)
ABS_ACT_DX = (0, 1)
PT_POOL_DX = (0, 1, 2)
PT_POOL_EXTRA_DX = 3      # dx 3 goes to Pool on odd halves only


def _dram_ap(t, offset, dims):
    return AP(tensor=t[:].tensor, offset=offset, ap=dims)


def _elide_covered_waits(nc, drop_act_self=True):
    """Drop sem waits already covered by an earlier wait on the same engine.

    Engines execute their instruction streams in order and Tile semaphores
    only increase, so a wait for (sem, value) when an earlier instruction on
    the same engine already waited for (sem, value' >= value) is a no-op.
    Tile's sem assignment emits these redundant waits at PSUM slot-reuse
    boundaries, where they overflow the LDWEIGHTS format's 1-wait budget.
    """
    skip = ("InstISA", "InstCustomDveAnt", "InstEventSemaphore")
    for b in nc.m.functions[0].blocks:
        seen = {}
        for ins in b.instructions:
            si = ins.sync_info
            if si is None or type(ins).__name__ in skip:
                continue
            w = si.on_wait
            if not w:
                continue
            eng = str(ins.engine).split(".")[-1]
            em = seen.setdefault(eng, {})
            kept = []
            for x in w:
                monotone = x.ant_name is not None and x.ant_name.startswith(
                    ("PE_", "DVE_", "Activation_", "Pool_", "SP_",
                     "DMAHW", "DMASW"))
                if (x.wait_mode != "sem-ge-imm" or x.wait_value is None
                        or x.wait_reg is not None or not monotone):
                    kept.append(x)
                    continue
                # ACT's same-engine self-waits are redundant (in-order
                # engine, writes cannot overtake) — but only drop them when
                # the instruction carries other waits too, to stay within
                # the 1-wait format budget without tripping CoreSim's
                # strict same-engine RAW detector on solo self-waits.
                if (drop_act_self and eng == "Activation" and len(w) > 1
                        and x.ant_name.startswith(eng + "_")):
                    continue
                if em.get(x.ant_name, -1) >= x.wait_value:
                    continue
                kept.append(x)
                em[x.ant_name] = x.wait_value
            if len(kept) != len(w):
                si.on_wait = kept


def _split_excess_waits(nc):
    """Move all-but-one sync waits onto injected same-engine NoOps.

    TPB compute instruction formats encode a single embedded wait command
    (walrus rejects more with "Too many sync wait commands"). A NoOp on the
    same in-order engine carrying the extra waits immediately before the
    instruction is semantically identical: the engine stalls at the nop
    until the semaphores reach their thresholds, then proceeds.
    """
    n = 0
    skip = ("InstISA", "InstCustomDveAnt", "InstEventSemaphore")
    for b in nc.m.functions[0].blocks:
        out = []
        changed = False
        for ins in b.instructions:
            si = ins.sync_info
            w = list(si.on_wait) if si is not None and si.on_wait else []
            if len(w) > 1 and type(ins).__name__ not in skip:
                for x in w[:-1]:
                    n += 1
                    nop = mybir.InstNoOp(name=f"I-waitnop-{n}",
                                         engine=ins.engine)
                    nop.sync_info = mybir.SyncInfo(on_wait=[x], on_update=[])
                    try:
                        nop.debug = ins.debug
                    except Exception:
                        pass
                    out.append(nop)
                si.on_wait = [w[-1]]
                changed = True
            out.append(ins)
        if changed:
            b.instructions = out
    return nc


def build_program(beta: float, split_waits: bool = True):
    nc = bass.Bass()
    xs = nc.declare_dram_parameter("xs", [SL, H + 4, W + 4], F32, isOutput=False)
    # lam[:, :, 0] = ln(sigmoid(mix)), lam[:, :, 1] = 1 - sigmoid(mix)
    lam = nc.declare_dram_parameter("lam", [NT, 128, 2], F32, isOutput=False)
    ident = nc.declare_dram_parameter("ident", [128, 128], BF16, isOutput=False)
    zeros = nc.declare_dram_parameter("zeros", [128, 512], BF16, isOutput=False)
    y = nc.declare_dram_parameter("y", [SL, H, W], F32, isOutput=True)

    HW = H * W
    HPWP = (H + 4) * WP
    XFD = RP * WP            # 3120 f32 elements per partition of an X tile

    with TileContext(nc) as tc:
        with (
            tc.tile_pool(name="consts", bufs=1) as cpool,
            tc.tile_pool(name="main", bufs=2) as pool,
            tc.tile_pool(name="single", bufs=1) as spool,
            tc.psum_pool(name="ps", bufs=2) as pspool,
        ):
            itile = cpool.tile([128, 128], BF16)
            nc.sync.dma_start(out=itile[:], in_=ident[:])
            ztile = cpool.tile([128, 512], BF16)
            nc.sync.dma_start(out=ztile[:], in_=zeros[:])
            cbias = cpool.tile([128, 1], F32)
            nc.vector.memset(cbias[:], C_BIAS)

            psum_releases = {}
            for t in range(NT):
                # ---- load big tile t (4 slices x 32 strips) in one DMA ----
                X = pool.tile([128, RP, WP], F32, tag="X")
                nc.sync.dma_start(
                    out=X[:],
                    in_=_dram_ap(xs, t * SPT * HPWP,
                                 [[HPWP, SPT], [R * WP, NSTRIP], [1, RP * WP]]),
                )
                lamt = pool.tile([128, 2], F32, tag="lam")
                nc.sync.dma_start(out=lamt[:], in_=lam[t])

                # ---- f32 -> bf16 converts on ACT (Xbf1 = Xbf shifted by one
                # column so odd-dx window reads stay 4-byte aligned for DVE
                # 2x mode) ----
                Xflat = X[:].rearrange("p a b -> p (a b)")
                Xbf = pool.tile([128, RP, WP], BF16, tag="Xbf")
                nc.scalar.activation(Xbf[:].rearrange("p a b -> p (a b)"),
                                     Xflat, AF.Copy)
                Xbf1 = pool.tile([128, RP, WP], BF16, tag="Xbf1")
                Xbf1_f = Xbf1[:].rearrange("p a b -> p (a b)")
                nc.scalar.activation(Xbf1_f[:, 0:XFD - 1], Xflat[:, 1:XFD],
                                     AF.Copy)

                # ---- separable 25-sum (for the mean) in f32 on DVE ----
                V = spool.tile([128, R, WP], F32, tag="V")
                nc.vector.tensor_tensor(V[:], X[:, 0:R, :], X[:, 1:R + 1, :], ALU.add)
                for dy in (2, 3, 4):
                    nc.vector.tensor_tensor(V[:], V[:], X[:, dy:dy + R, :], ALU.add)
                M = spool.tile([128, R, W], F32, tag="M")
                nc.vector.tensor_tensor(M[:], V[:, :, 0:W], V[:, :, 1:W + 1], ALU.add)
                for dx in (2, 3):
                    nc.vector.tensor_tensor(M[:], M[:], V[:, :, dx:dx + W], ALU.add)
                # last add folds in nothing extra; mus = -M/25 in bf16
                nc.vector.tensor_tensor(M[:], M[:], V[:, :, 4:4 + W], ALU.add)
                mus = spool.tile([128, R, W], BF16, tag="mus")
                nc.vector.tensor_scalar(mus[:], M[:], -1.0 / 25.0, None, ALU.mult)

                # ---- separable 5x5 max (DVE, bf16 2x) ----
                MV = spool.tile([128, R, WP], BF16, tag="MV")
                nc.vector.tensor_tensor(MV[:], Xbf[:, 0:R, :], Xbf[:, 1:R + 1, :],
                                        ALU.max)
                for dy in (2, 3, 4):
                    nc.vector.tensor_tensor(MV[:], MV[:], Xbf[:, dy:dy + R, :],
                                            ALU.max)
                MX = spool.tile([128, R, W], BF16, tag="MX")
                nc.vector.tensor_tensor(MX[:], MV[:, :, 0:W], MV[:, :, 1:W + 1],
                                        ALU.max)
                for dx in (2, 3, 4):
                    nc.vector.tensor_tensor(MX[:], MX[:], MV[:, :, dx:dx + W],
                                            ALU.max)
                # MXw = (1-lam) * max, f32, on DVE (cheap 2x single-src)
                MXw = spool.tile([128, R, W], F32, tag="MXw")
                nc.vector.tensor_scalar(MXw[:], MX[:], lamt[:, 1:2], None,
                                        ALU.mult)

                # ---- 25 taps: five dy-merged units per dx, per half ----
                for h in range(2):
                    r0 = h * HR
                    g = 2 * t + h
                    S_ps = pspool.tile([128, HR, W], F32, tag="S")
                    T_ps = pspool.tile([128, HR, W], F32, tag="T")
                    # A PE nop (CTRL format: multi-wait budget) syncs on the
                    # recycled slot's readers so the warmup matmuls keep
                    # only their single PE-drain wait.
                    if g >= 2:
                        pe_nop = nc.tensor.nop()
                        for rel in psum_releases[g - 2]:
                            add_dep_helper(pe_nop.ins, rel.ins, sync=True,
                                           reason="psum slot reader sync")
                    else:
                        pe_nop = None
                    for cch in range(2):
                        sel = (slice(None), slice(2 * cch, 2 * cch + 2),
                               slice(None))
                        wm_t = nc.tensor.matmul(T_ps[sel], itile[:], ztile[:],
                                                start=True, stop=False)
                        wm_s = nc.tensor.matmul(S_ps[sel], itile[:], ztile[:],
                                                start=True, stop=False)
                        if pe_nop is not None:
                            add_dep_helper(wm_t.ins, pe_nop.ins, sync=True,
                                           reason="nop before warmup")
                            add_dep_helper(wm_s.ins, pe_nop.ins, sync=True,
                                           reason="nop before warmup")

                    # broadcast AP: -mu repeated for the 5 dy taps
                    mus5 = AP(tensor=mus[:].tensor,
                              offset=mus[:].offset + r0 * W,
                              ap=[list(mus[:].ap[0]), [0, 5], [W, HR], [1, W]])

                    for dx in range(5):
                        if dx % 2 == 0:
                            src, c0 = Xbf, dx
                        else:
                            src, c0 = Xbf1, dx - 1
                        # [128, 5(dy), HR, W] window view, rows r0+dy..
                        xw5 = AP(tensor=src[:].tensor,
                                 offset=src[:].offset + r0 * WP + c0,
                                 ap=[list(src[:].ap[0]), [WP, 5], [WP, HR],
                                     [1, W]])
                        d5 = pool.tile([128, 5, HR, W], BF16, tag="d5")
                        nc.vector.tensor_tensor(d5[:], xw5, mus5, ALU.add)
                        # abs: clear bf16 sign bits pairwise via int32 view
                        if dx in ABS_ACT_DX:
                            nc.scalar.activation(d5[:], d5[:], AF.Abs)
                        else:
                            di = d5[:].rearrange(
                                "p a b c -> p (a b c)").bitcast(I32)
                            nc.vector.tensor_scalar(di, di, 0x7FFF7FFF, None,
                                                    ALU.bitwise_and)
                        st5 = pool.tile([128, 5, HR, W], BF16, tag="st5")
                        nc.scalar.activation(st5[:], d5[:], AF.Exp,
                                             bias=cbias[:], scale=-beta)
                        for dy in range(5):
                            last = dx == 4 and dy == 4
                            for cch in range(2):
                                sel = (slice(None),
                                       slice(2 * cch, 2 * cch + 2),
                                       slice(None))
                                nc.tensor.matmul(
                                    S_ps[sel], itile[:],
                                    st5[:, dy, 2 * cch:2 * cch + 2, :],
                                    start=False, stop=last)
                        pt5 = pool.tile([128, 5, HR, W], BF16, tag="pt5")
                        on_pool = dx in PT_POOL_DX or (
                            dx == PT_POOL_EXTRA_DX and g % 2 == 1)
                        eng = nc.gpsimd if on_pool else nc.vector
                        eng.tensor_tensor(pt5[:], st5[:], xw5, ALU.mult)
                        for dy in range(5):
                            last = dx == 4 and dy == 4
                            for cch in range(2):
                                sel = (slice(None),
                                       slice(2 * cch, 2 * cch + 2),
                                       slice(None))
                                nc.tensor.matmul(
                                    T_ps[sel], itile[:],
                                    pt5[:, dy, 2 * cch:2 * cch + 2, :],
                                    start=False, stop=last)

                    # ---- combine: out = lam*(T/S) + (1-lam)*max ----
                    # lam/S = exp(-ln S + ln lam) in one ACT op.
                    lnS = spool.tile([128, HR, W], F32, tag="lnS")
                    rel_s = nc.scalar.activation(lnS[:], S_ps[:], AF.Ln)
                    rSl = spool.tile([128, HR, W], F32, tag="rSl")
                    nc.scalar.activation(rSl[:], lnS[:], AF.Exp, scale=-1.0,
                                         bias=lamt[:, 0:1])
                    med = spool.tile([128, HR, W], F32, tag="med")
                    rel_t = nc.vector.tensor_tensor(med[:], rSl[:], T_ps[:],
                                                    ALU.mult)
                    psum_releases[g] = (rel_s, rel_t)
                    out_t = pool.tile([128, HR, W], F32, tag="out")
                    nc.vector.tensor_tensor(out_t[:], med[:],
                                            MXw[:, r0:r0 + HR, :], ALU.add)

                    # ---- store half-tile ----
                    nc.sync.dma_start(
                        out=_dram_ap(y, t * SPT * HW + h * HR * W,
                                     [[HW, SPT], [R * W, NSTRIP],
                                      [1, HR * W]]),
                        in_=out_t[:],
                    )
    _elide_covered_waits(nc, drop_act_self=split_waits)
    if split_waits:
        # Mechanical transform for walrus's 1-wait instruction formats;
        # skip under CoreSim (its race detector requires sem updates on
        # every instruction, which the injected bare NoOps lack).
        _split_excess_waits(nc)
    return nc


def _make_inputs(x, mix, beta_raw):
    """Host-side sharding. Returns (beta, in_maps)."""
    bf16 = mybir.dt.np(BF16)
    x = np.ascontiguousarray(x, dtype=np.float32)
    mix = np.asarray(mix, dtype=np.float32).reshape(C)
    beta_raw = float(np.asarray(beta_raw, dtype=np.float32))
    beta = float(5.0 + 45.0 / (1.0 + np.exp(-beta_raw)))
    lam_c = 1.0 / (1.0 + np.exp(-mix.astype(np.float64)))

    xs_all = np.pad(x.reshape(B * C, H, W), ((0, 0), (2, 2), (2, 2)),
                    mode="reflect")
    ident = np.eye(128, dtype=bf16)
    zeros = np.zeros((128, 512), dtype=bf16)
    in_maps = []
    for core in range(NCORES):
        sl0 = core * SL
        shard = np.ascontiguousarray(xs_all[sl0:sl0 + SL])
        lam_t = np.empty((NT, 128, 2), dtype=np.float32)
        for t in range(NT):
            for p in range(128):
                g_slice = sl0 + t * SPT + p // NSTRIP
                lc = lam_c[g_slice % C]
                lam_t[t, p, 0] = np.log(lc)
                lam_t[t, p, 1] = 1.0 - lc
        in_maps.append({"xs": shard, "lam": lam_t, "ident": ident,
                        "zeros": zeros})
    return beta, in_maps


def kernel(x, mix, beta_raw):
    beta, in_maps = _make_inputs(x, mix, beta_raw)
    nc = build_program(beta)
    res = run_bass_kernel_spmd(nc, in_maps, list(range(NCORES))).results
    out = np.concatenate([res[i]["y"].reshape(SL, H, W) for i in range(NCORES)],
                         axis=0)
    return np.ascontiguousarray(out.reshape(B, C, H, W))
